# revision 19
# baseline (speedup 1.0000x reference)
# Trainium2 Bass kernel for BlipAttention (B=4, S=2048, D=768, H=12, HD=64).
#
# Sharding: 8 cores = (batch b in 0..3) x (head-group g in 0..1), 6 heads per
# core. Each core computes, for its (b, g):
#   mixedT = [qT; kT] = W_qk^T-contraction against hidden^T   (j on partitions)
#   v_nat  = hidden @ W_v^T                                   (s on partitions)
#   per head h: scoresT[sk, sq] = k_h q_h^T;  expT = exp(scale * scoresT)
#   ctxT_aug[d+1, sq] = [v_h; 1]^T @ expT   (row 64 = softmax denominator)
#   ctx_norm = ctxT / denom;  out_partial = ctx_norm^T @ Wp_g^T
# Host sums the two head-group partials per batch and adds proj_b.
#
# All matmuls run as float32r (full-rate fp32 path, N>=256). exp runs on
# ScalarE reading PSUM in 3-bank groups to amortize the per-op overhead.

import ml_dtypes
import numpy as np

import concourse.bass as bass
from concourse import bacc
import concourse.mybir as mybir
import concourse.tile as tile

F32 = mybir.dt.float32
F32R = mybir.dt.float32r
BF16 = mybir.dt.bfloat16

D = 768
HD = 64
HL = 6  # heads per core
DL = HL * HD  # 384: local d-span of this head group
SCALE = HD**-0.5
P = 128
KD = D // P  # 6 k-tiles over the model dim


def _pack(mat_T, p):
    """[R, C] -> [p, R//p, C] SBUF-layout pack (row r = kd*p + part)."""
    R, C = mat_T.shape
    return np.ascontiguousarray(
        mat_T.reshape(R // p, p, C).transpose(1, 0, 2)
    ).astype(ml_dtypes.bfloat16)


def _emit(tc, S, hT, wqkT, wvT, bqk, bv, wpT, vones, out):
    """Emit the per-core program. All APs are DRAM tensors.

    hT   [D, S]      hidden^T for this batch
    wqkT [D, 2*DL]   cols 0:DL -> q weights (d_model x j), DL:2DL -> k
    wvT  [D, DL]
    bqk  [P, 2*DL//P] qkv bias for q,k rows in [p, jt] layout
    bv   [DL]
    wpT  [DL, D]     proj weight slice, [d_local, e]
    out  [S, D]
    """
    nc = tc.nc
    import contextlib

    NT = S // P  # sk tiles
    SQC = min(512, S)  # sq chunk width
    NSQC = S // SQC
    JT = 2 * DL // P  # 6 j-tiles for q,k
    G = 3  # sk tiles per exp group (3 PSUM banks)

    groups = [(s, min(s + G, NT)) for s in range(0, NT, G)]

    with contextlib.ExitStack() as ctx:
        pool_qkv = ctx.enter_context(tc.tile_pool(name="pool_qkv", bufs=1))

        qkT_sb = pool_qkv.tile([P, JT, S], BF16)
        v_sb = pool_qkv.tile([P, NT, HL * (HD + 1)], BF16)

        with contextlib.ExitStack() as wctx:
            pool_w = wctx.enter_context(tc.tile_pool(name="pool_w", bufs=1))
            ps_qkv = wctx.enter_context(
                tc.tile_pool(name="ps_qkv", bufs=2, space="PSUM")
            )

            # ---- load inputs --------------------------------------------
            # host pre-packs everything in SBUF layout; alternate the two
            # HWDGE queues (sync/scalar) for parallel contiguous transfers
            hT_sb = pool_w.tile([P, KD, S], BF16)
            wqkT_sb = pool_w.tile([P, KD, 2 * DL], BF16)
            wvT_sb = pool_w.tile([P, KD, DL], BF16)
            qs = [nc.sync, nc.scalar]
            for kd in range(KD):
                qs[kd % 2].dma_start(hT_sb[:, kd, :], hT[:, kd, :])
            for kd in range(0, KD, 3):
                qs[(kd // 3) % 2].dma_start(
                    wqkT_sb[:, kd : kd + 3, :], wqkT[:, kd : kd + 3, :]
                )
            nc.sync.dma_start(wvT_sb, wvT)
            bqk_sb = pool_w.tile([P, JT], F32)
            nc.scalar.dma_start(bqk_sb, bqk)
            bv_sb = pool_w.tile([P, DL], F32)
            nc.gpsimd.dma_start(
                bv_sb, bv.rearrange("d -> () d").to_broadcast([P, DL])
            )

            # ---- QKV projection ------------------------------------------
            # qkT_sb[p, jt, s]: jt 0..2 = q (j = jt*128+p), jt 3..5 = k
            for jt in range(JT):
                for sc in range(NSQC):
                    ps = ps_qkv.tile([P, SQC], F32, tag="qk")
                    for kd in range(KD):
                        nc.tensor.matmul(
                            ps,
                            lhsT=wqkT_sb[:, kd, jt * P : (jt + 1) * P],
                            rhs=hT_sb[:, kd, sc * SQC : (sc + 1) * SQC],
                            start=(kd == 0),
                            stop=(kd == KD - 1),
                        )
                    nc.vector.tensor_scalar_add(
                        out=qkT_sb[:, jt, sc * SQC : (sc + 1) * SQC],
                        in0=ps,
                        scalar1=bqk_sb[:, jt : jt + 1],
                    )

            # v_sb[p, st, h*65 .. h*65+64] = v columns, col h*65+64 = ones
            # ones into column HD of each head slot (denominator column);
            # DMA-broadcast from a tiny DRAM ones vector (f32r producer)
            v_view = v_sb.rearrange("p t (h x) -> p t h x", x=HD + 1)
            nc.gpsimd.dma_start(
                v_view[:, :, :, HD],
                vones.rearrange("(t h) -> () t h", t=NT).to_broadcast(
                    [P, NT, HL]
                ),
            )
            for st in range(NT):
                ps = ps_qkv.tile([P, DL], F32, tag="v")
                for kd in range(KD):
                    nc.tensor.matmul(
                        ps,
                        lhsT=hT_sb[:, kd, st * P : (st + 1) * P],
                        rhs=wvT_sb[:, kd, :],
                        start=(kd == 0),
                        stop=(kd == KD - 1),
                    )
                nc.vector.tensor_tensor(
                    out=v_view[:, st, :, 0:HD],
                    in0=ps.rearrange("p (h x) -> p h x", x=HD),
                    in1=bv_sb.rearrange("p (h x) -> p h x", x=HD),
                    op=mybir.AluOpType.add,
                )

        # wpT + ctxn allocate after the phase-1 pools release (SBUF budget)
        pool_keep = ctx.enter_context(tc.tile_pool(name="pool_keep", bufs=1))
        wpT_sb = pool_keep.tile([HD, HL, D], BF16)
        nc.sync.dma_start(wpT_sb, wpT)
        # proj lhsT lives here: [d_local(64), h, s]
        ctxn_sb = pool_keep.tile([HD, HL, S], BF16)

        # ---- attention ---------------------------------------------------
        with contextlib.ExitStack() as actx:
            ps_sc = actx.enter_context(
                tc.tile_pool(name="ps_sc", bufs=2, space="PSUM")
            )
            ps_ctx = actx.enter_context(
                tc.tile_pool(name="ps_ctx", bufs=2, space="PSUM")
            )
            pool_e = actx.enter_context(tc.tile_pool(name="pool_e", bufs=2))
            pool_cu = actx.enter_context(tc.tile_pool(name="pool_cu", bufs=2))
            pool_d = actx.enter_context(tc.tile_pool(name="pool_d", bufs=2))
            pool_dr = actx.enter_context(
                tc.tile_pool(name="pool_dr", bufs=2, space="DRAM")
            )

            for h in range(HL):
                hp = (h % 2) * HD  # base partition of this head in qkT_sb
                qjt = h // 2
                kjt = JT // 2 + h // 2
                ctxu = pool_cu.tile([HD + 1, NSQC, SQC], F32)
                for sc in range(NSQC):
                    ctx_ps = ps_ctx.tile([HD + 1, SQC], F32)
                    for g0, g1 in groups:
                        gl = g1 - g0
                        sc_ps = ps_sc.tile([P, G, SQC], F32)
                        for j in range(gl):
                            t = g0 + j
                            nc.tensor.matmul(
                                sc_ps[:, j, :],
                                lhsT=qkT_sb[
                                    hp : hp + HD, kjt, t * P : (t + 1) * P
                                ],
                                rhs=qkT_sb[
                                    hp : hp + HD, qjt, sc * SQC : (sc + 1) * SQC
                                ],
                                start=True,
                                stop=True,
                            )
                        e_sb = pool_e.tile([P, G, SQC], BF16)
                        nc.scalar.activation(
                            out=e_sb[:, 0:gl, :],
                            in_=sc_ps[:, 0:gl, :],
                            func=mybir.ActivationFunctionType.Exp,
                            scale=float(SCALE),
                        )
                        for j in range(gl):
                            t = g0 + j
                            nc.tensor.matmul(
                                ctx_ps,
                                lhsT=v_sb[
                                    :, t, h * (HD + 1) : (h + 1) * (HD + 1)
                                ],
                                rhs=e_sb[:, j, :],
                                start=(t == 0),
                                stop=(t == NT - 1),
                            )
                    nc.vector.tensor_copy(ctxu[:, sc, :], ctx_ps)

                # denominators: row HD of ctxu, [1, S] -> packed [S/128, 128]
                npk = S // P
                dpack = pool_d.tile([npk, P], F32, tag="dpack")
                nc.sync.dma_start(
                    dpack,
                    ctxu[HD : HD + 1, :, :].rearrange("o a s -> o (a s)"),
                )
                rpack = pool_d.tile([npk, P], F32, tag="rpack")
                nc.vector.reciprocal(rpack, dpack)
                rdram = pool_dr.tile([S], F32, tag="rdram")
                nc.sync.dma_start(rdram.rearrange("(x y) -> x y", y=P), rpack)
                rbc = pool_d.tile([HD, S], F32, tag="rbc")
                nc.gpsimd.dma_start(
                    rbc, rdram.rearrange("s -> () s").to_broadcast([HD, S])
                )
                for sc in range(NSQC):
                    nc.vector.tensor_tensor(
                        out=ctxn_sb[:, h, sc * SQC : (sc + 1) * SQC],
                        in0=ctxu[0:HD, sc, :],
                        in1=rbc[:, sc * SQC : (sc + 1) * SQC],
                        op=mybir.AluOpType.mult,
                    )

        # ---- output projection -------------------------------------------
        with contextlib.ExitStack() as pctx:
            ps_prj = pctx.enter_context(
                tc.tile_pool(name="ps_prj", bufs=3, space="PSUM")
            )
            pool_o = pctx.enter_context(tc.tile_pool(name="pool_o", bufs=3))
            EC = D // 2  # 384-wide chunks (psum bank fp32 limit 512)
            for st in range(NT):
                o_sb = pool_o.tile([P, D], F32)
                for ec in range(2):
                    ps = ps_prj.tile([P, EC], F32)
                    for h in range(HL):
                        nc.tensor.matmul(
                            ps,
                            lhsT=ctxn_sb[:, h, st * P : (st + 1) * P],
                            rhs=wpT_sb[:, h, ec * EC : (ec + 1) * EC],
                            start=(h == 0),
                            stop=(h == HL - 1),
                        )
                    nc.vector.tensor_copy(o_sb[:, ec * EC : (ec + 1) * EC], ps)
                    oq = [nc.sync, nc.scalar][(st * 2 + ec) % 2]
                    oq.dma_start(
                        out[st * P : (st + 1) * P, ec * EC : (ec + 1) * EC],
                        o_sb[:, ec * EC : (ec + 1) * EC],
                    )


_NC_CACHE = {}


def _build(S=2048):
    if S in _NC_CACHE:
        return _NC_CACHE[S]
    nc = bacc.Bacc("TRN2", target_bir_lowering=False, debug=False)
    hT = nc.dram_tensor("hT", [P, KD, S], BF16, kind="ExternalInput").ap()
    wqkT = nc.dram_tensor("wqkT", [P, KD, 2 * DL], BF16, kind="ExternalInput").ap()
    wvT = nc.dram_tensor("wvT", [P, KD, DL], BF16, kind="ExternalInput").ap()
    bqk = nc.dram_tensor("bqk", [P, 2 * DL // P], F32, kind="ExternalInput").ap()
    bv = nc.dram_tensor("bv", [DL], F32, kind="ExternalInput").ap()
    wpT = nc.dram_tensor("wpT", [HD, HL, D], BF16, kind="ExternalInput").ap()
    vones = nc.dram_tensor(
        "vones", [(S // P) * HL], BF16, kind="ExternalInput"
    ).ap()
    out = nc.dram_tensor("out", [S, D], F32, kind="ExternalOutput").ap()
    with tile.TileContext(nc) as tc:
        _emit(tc, S, hT, wqkT, wvT, bqk, bv, wpT, vones, out)
    nc.compile()
    _NC_CACHE[S] = nc
    return nc


def shard_inputs(hidden_states, qkv_w, qkv_b, proj_w, S=2048):
    """Build the 8 per-core input maps (numpy only)."""
    hidden_states = np.asarray(hidden_states, dtype=np.float32)
    qkv_w = np.asarray(qkv_w, dtype=np.float32)
    qkv_b = np.asarray(qkv_b, dtype=np.float32)
    proj_w = np.asarray(proj_w, dtype=np.float32)
    in_maps = []
    for c in range(8):
        b, g = divmod(c, 2)
        qs = slice(g * DL, (g + 1) * DL)
        ks = slice(D + g * DL, D + (g + 1) * DL)
        vs = slice(2 * D + g * DL, 2 * D + (g + 1) * DL)
        wq = qkv_w[qs]  # [DL, D]
        wk = qkv_w[ks]
        wv = qkv_w[vs]
        bqk = np.concatenate([qkv_b[qs], qkv_b[ks]])  # [2*DL]
        in_maps.append(
            {
                "hT": _pack(hidden_states[b].T, P),
                "wqkT": _pack(np.concatenate([wq, wk], axis=0).T, P),
                "wvT": _pack(wv.T, P),
                "bqk": np.ascontiguousarray(
                    bqk.reshape(2 * DL // P, P).T
                ),
                "bv": np.ascontiguousarray(qkv_b[vs]),
                "wpT": _pack(proj_w[:, g * DL : (g + 1) * DL].T, HD),
                "vones": np.ones(((S // 128) * HL,), dtype=ml_dtypes.bfloat16),
            }
        )
    return in_maps


def kernel(hidden_states, qkv_w, qkv_b, proj_w, proj_b):
    from concourse.bass_utils import run_bass_kernel_spmd

    hidden_states = np.asarray(hidden_states, dtype=np.float32)
    proj_b = np.asarray(proj_b, dtype=np.float32)
    B, S, d = hidden_states.shape
    in_maps = shard_inputs(hidden_states, qkv_w, qkv_b, proj_w, S=S)
    nc = _build(S)
    res = run_bass_kernel_spmd(nc, in_maps, core_ids=list(range(8)))
    parts = [r["out"] for r in res.results]
    out = np.stack(
        [parts[2 * b] + parts[2 * b + 1] + proj_b[None, :] for b in range(B)]
    )
    return out.astype(np.float32)


# revision 20
# speedup vs baseline: 1.2137x; 1.2137x over previous
# Trainium2 Bass kernel for BlipAttention (B=4, S=2048, D=768, H=12, HD=64).
#
# Sharding: 8 cores = (batch b in 0..3) x (head-group g in 0..1), 6 heads per
# core. Each core computes, for its (b, g):
#   mixedT = [qT; kT] = W_qk^T-contraction against hidden^T   (j on partitions)
#   v_nat  = hidden @ W_v^T                                   (s on partitions)
#   per head h: scoresT[sk, sq] = k_h q_h^T;  expT = exp(scale * scoresT)
#   ctxT_aug[d+1, sq] = [v_h; 1]^T @ expT   (row 64 = softmax denominator)
#   ctx_norm = ctxT / denom;  out_partial = ctx_norm^T @ Wp_g^T
# Host sums the two head-group partials per batch and adds proj_b.
#
# All matmuls run as float32r (full-rate fp32 path, N>=256). exp runs on
# ScalarE reading PSUM in 3-bank groups to amortize the per-op overhead.

import ml_dtypes
import numpy as np

import concourse.bass as bass
from concourse import bacc
import concourse.mybir as mybir
import concourse.tile as tile

F32 = mybir.dt.float32
F32R = mybir.dt.float32r
BF16 = mybir.dt.bfloat16

D = 768
HD = 64
HL = 6  # heads per core
DL = HL * HD  # 384: local d-span of this head group
SCALE = HD**-0.5
P = 128
KD = D // P  # 6 k-tiles over the model dim


def _pack(mat_T, p):
    """[R, C] -> [p, R//p, C] SBUF-layout pack (row r = kd*p + part)."""
    R, C = mat_T.shape
    return np.ascontiguousarray(
        mat_T.reshape(R // p, p, C).transpose(1, 0, 2)
    ).astype(ml_dtypes.bfloat16)


def _emit(tc, S, hT, wqkT, wvT, bqk, bv, wpT, vones, o64, out):
    """Emit the per-core program. All APs are DRAM tensors.

    hT   [D, S]      hidden^T for this batch
    wqkT [D, 2*DL]   cols 0:DL -> q weights (d_model x j), DL:2DL -> k
    wvT  [D, DL]
    bqk  [P, 2*DL//P] qkv bias for q,k rows in [p, jt] layout
    bv   [DL]
    wpT  [DL, D]     proj weight slice, [d_local, e]
    out  [S, D]
    """
    nc = tc.nc
    import contextlib

    NT = S // P  # sk tiles
    SQC = min(512, S)  # sq chunk width
    NSQC = S // SQC
    JT = 2 * DL // P  # 6 j-tiles for q,k
    G = 3  # sk tiles per exp group (3 PSUM banks)

    groups = [(s, min(s + G, NT)) for s in range(0, NT, G)]

    with contextlib.ExitStack() as ctx:
        pool_qkv = ctx.enter_context(tc.tile_pool(name="pool_qkv", bufs=1))

        qkT_sb = pool_qkv.tile([P, JT, S], BF16)
        v_sb = pool_qkv.tile([P, NT, HL * (HD + 1)], BF16)
        ones64 = pool_qkv.tile([1, HD], F32)
        nc.sync.dma_start(ones64, o64)

        with contextlib.ExitStack() as wctx:
            pool_w = wctx.enter_context(tc.tile_pool(name="pool_w", bufs=1))
            ps_qkv = wctx.enter_context(
                tc.tile_pool(name="ps_qkv", bufs=2, space="PSUM")
            )

            # ---- load inputs --------------------------------------------
            # host pre-packs everything in SBUF layout; alternate the two
            # HWDGE queues (sync/scalar) for parallel contiguous transfers
            hT_sb = pool_w.tile([P, KD, S], BF16)
            wqkT_sb = pool_w.tile([P, KD, 2 * DL], BF16)
            wvT_sb = pool_w.tile([P, KD, DL], BF16)
            qs = [nc.sync, nc.scalar]
            for kd in range(KD):
                qs[kd % 2].dma_start(hT_sb[:, kd, :], hT[:, kd, :])
            for kd in range(0, KD, 3):
                qs[(kd // 3) % 2].dma_start(
                    wqkT_sb[:, kd : kd + 3, :], wqkT[:, kd : kd + 3, :]
                )
            nc.sync.dma_start(wvT_sb, wvT)
            bqk_sb = pool_w.tile([P, JT], F32)
            nc.scalar.dma_start(bqk_sb, bqk)
            bv_sb = pool_w.tile([P, DL], F32)
            nc.sync.dma_start(bv_sb, bv)

            # ---- QKV projection ------------------------------------------
            # qkT_sb[p, jt, s]: jt 0..2 = q (j = jt*128+p), jt 3..5 = k
            for jt in range(JT):
                for sc in range(NSQC):
                    ps = ps_qkv.tile([P, SQC], F32, tag="qk")
                    for kd in range(KD):
                        nc.tensor.matmul(
                            ps,
                            lhsT=wqkT_sb[:, kd, jt * P : (jt + 1) * P],
                            rhs=hT_sb[:, kd, sc * SQC : (sc + 1) * SQC],
                            start=(kd == 0),
                            stop=(kd == KD - 1),
                        )
                    nc.vector.tensor_scalar_add(
                        out=qkT_sb[:, jt, sc * SQC : (sc + 1) * SQC],
                        in0=ps,
                        scalar1=bqk_sb[:, jt : jt + 1],
                    )

            # v_sb[p, st, h*65 .. h*65+64] = v columns, col h*65+64 = ones
            # (denominator column): clean 2D ones load + strided DVE copy
            v_view = v_sb.rearrange("p t (h x) -> p t h x", x=HD + 1)
            ones_sb = pool_w.tile([P, NT * HL], BF16)
            nc.scalar.dma_start(ones_sb, vones)
            nc.vector.tensor_copy(
                v_view[:, :, :, HD],
                ones_sb.rearrange("p (t h) -> p t h", t=NT),
            )
            for st in range(NT):
                ps = ps_qkv.tile([P, DL], F32, tag="v")
                for kd in range(KD):
                    nc.tensor.matmul(
                        ps,
                        lhsT=hT_sb[:, kd, st * P : (st + 1) * P],
                        rhs=wvT_sb[:, kd, :],
                        start=(kd == 0),
                        stop=(kd == KD - 1),
                    )
                nc.vector.tensor_tensor(
                    out=v_view[:, st, :, 0:HD],
                    in0=ps.rearrange("p (h x) -> p h x", x=HD),
                    in1=bv_sb.rearrange("p (h x) -> p h x", x=HD),
                    op=mybir.AluOpType.add,
                )

        # wpT + ctxn allocate after the phase-1 pools release (SBUF budget)
        pool_keep = ctx.enter_context(tc.tile_pool(name="pool_keep", bufs=1))
        wpT_sb = pool_keep.tile([HD, HL, D], BF16)
        nc.sync.dma_start(wpT_sb, wpT)
        # proj lhsT lives here: [d_local(64), h, s]
        ctxn_sb = pool_keep.tile([HD, HL, S], BF16)

        # ---- attention ---------------------------------------------------
        with contextlib.ExitStack() as actx:
            ps_sc = actx.enter_context(
                tc.tile_pool(name="ps_sc", bufs=2, space="PSUM")
            )
            ps_ctx = actx.enter_context(
                tc.tile_pool(name="ps_ctx", bufs=1, space="PSUM")
            )
            ps_bc = actx.enter_context(
                tc.tile_pool(name="ps_bc", bufs=1, space="PSUM")
            )
            pool_e = actx.enter_context(tc.tile_pool(name="pool_e", bufs=2))
            pool_cu = actx.enter_context(tc.tile_pool(name="pool_cu", bufs=2))
            pool_d = actx.enter_context(tc.tile_pool(name="pool_d", bufs=2))
            pool_dr = actx.enter_context(
                tc.tile_pool(name="pool_dr", bufs=2, space="DRAM")
            )

            for h in range(HL):
                hp = (h % 2) * HD  # base partition of this head in qkT_sb
                qjt = h // 2
                kjt = JT // 2 + h // 2
                ctxu = pool_cu.tile([HD + 1, NSQC, SQC], F32)
                for sc in range(NSQC):
                    ctx_ps = ps_ctx.tile([HD + 1, SQC], F32)
                    for g0, g1 in groups:
                        gl = g1 - g0
                        sc_ps = ps_sc.tile([P, G, SQC], F32)
                        for j in range(gl):
                            t = g0 + j
                            nc.tensor.matmul(
                                sc_ps[:, j, :],
                                lhsT=qkT_sb[
                                    hp : hp + HD, kjt, t * P : (t + 1) * P
                                ],
                                rhs=qkT_sb[
                                    hp : hp + HD, qjt, sc * SQC : (sc + 1) * SQC
                                ],
                                start=True,
                                stop=True,
                            )
                        e_sb = pool_e.tile([P, G, SQC], BF16)
                        nc.scalar.activation(
                            out=e_sb[:, 0:gl, :],
                            in_=sc_ps[:, 0:gl, :],
                            func=mybir.ActivationFunctionType.Exp,
                            scale=float(SCALE),
                        )
                        for j in range(gl):
                            t = g0 + j
                            nc.tensor.matmul(
                                ctx_ps,
                                lhsT=v_sb[
                                    :, t, h * (HD + 1) : (h + 1) * (HD + 1)
                                ],
                                rhs=e_sb[:, j, :],
                                start=(t == 0),
                                stop=(t == NT - 1),
                            )
                    nc.vector.tensor_copy(ctxu[:, sc, :], ctx_ps)

                # denominators: row HD of ctxu, [1, S] -> packed [S/128, 128]
                npk = S // P
                dpack = pool_d.tile([npk, P], F32, tag="dpack")
                nc.sync.dma_start(
                    dpack,
                    ctxu[HD : HD + 1, :, :].rearrange("o a s -> o (a s)"),
                )
                rpack = pool_d.tile([npk, P], F32, tag="rpack")
                nc.vector.reciprocal(rpack, dpack)
                rdram = pool_dr.tile([S], F32, tag="rdram")
                nc.sync.dma_start(rdram.rearrange("(x y) -> x y", y=P), rpack)
                rrow = pool_d.tile([1, S], F32, tag="rrow")
                nc.scalar.dma_start(rrow, rdram.rearrange("s -> () s"))
                for sc in range(NSQC):
                    # broadcast recip across the 64 d-partitions via K=1 matmul
                    bc = ps_bc.tile([HD, SQC], F32)
                    nc.tensor.matmul(
                        bc,
                        lhsT=ones64[0:1, :],
                        rhs=rrow[0:1, sc * SQC : (sc + 1) * SQC],
                        start=True,
                        stop=True,
                    )
                    nc.vector.tensor_tensor(
                        out=ctxn_sb[:, h, sc * SQC : (sc + 1) * SQC],
                        in0=ctxu[0:HD, sc, :],
                        in1=bc,
                        op=mybir.AluOpType.mult,
                    )

        # ---- output projection -------------------------------------------
        with contextlib.ExitStack() as pctx:
            ps_prj = pctx.enter_context(
                tc.tile_pool(name="ps_prj", bufs=3, space="PSUM")
            )
            pool_o = pctx.enter_context(tc.tile_pool(name="pool_o", bufs=3))
            EC = D // 2  # 384-wide chunks (psum bank fp32 limit 512)
            for st in range(NT):
                o_sb = pool_o.tile([P, D], F32)
                for ec in range(2):
                    ps = ps_prj.tile([P, EC], F32)
                    for h in range(HL):
                        nc.tensor.matmul(
                            ps,
                            lhsT=ctxn_sb[:, h, st * P : (st + 1) * P],
                            rhs=wpT_sb[:, h, ec * EC : (ec + 1) * EC],
                            start=(h == 0),
                            stop=(h == HL - 1),
                        )
                    nc.vector.tensor_copy(o_sb[:, ec * EC : (ec + 1) * EC], ps)
                    oq = [nc.sync, nc.scalar][(st * 2 + ec) % 2]
                    oq.dma_start(
                        out[st * P : (st + 1) * P, ec * EC : (ec + 1) * EC],
                        o_sb[:, ec * EC : (ec + 1) * EC],
                    )


_NC_CACHE = {}


def _build(S=2048):
    if S in _NC_CACHE:
        return _NC_CACHE[S]
    nc = bacc.Bacc("TRN2", target_bir_lowering=False, debug=False)
    hT = nc.dram_tensor("hT", [P, KD, S], BF16, kind="ExternalInput").ap()
    wqkT = nc.dram_tensor("wqkT", [P, KD, 2 * DL], BF16, kind="ExternalInput").ap()
    wvT = nc.dram_tensor("wvT", [P, KD, DL], BF16, kind="ExternalInput").ap()
    bqk = nc.dram_tensor("bqk", [P, 2 * DL // P], F32, kind="ExternalInput").ap()
    bv = nc.dram_tensor("bv", [P, DL], F32, kind="ExternalInput").ap()
    wpT = nc.dram_tensor("wpT", [HD, HL, D], BF16, kind="ExternalInput").ap()
    vones = nc.dram_tensor(
        "vones", [P, (S // P) * HL], BF16, kind="ExternalInput"
    ).ap()
    o64 = nc.dram_tensor("o64", [1, HD], F32, kind="ExternalInput").ap()
    out = nc.dram_tensor("out", [S, D], F32, kind="ExternalOutput").ap()
    with tile.TileContext(nc) as tc:
        _emit(tc, S, hT, wqkT, wvT, bqk, bv, wpT, vones, o64, out)
    nc.compile()
    _NC_CACHE[S] = nc
    return nc


def shard_inputs(hidden_states, qkv_w, qkv_b, proj_w, S=2048):
    """Build the 8 per-core input maps (numpy only)."""
    hidden_states = np.asarray(hidden_states, dtype=np.float32)
    qkv_w = np.asarray(qkv_w, dtype=np.float32)
    qkv_b = np.asarray(qkv_b, dtype=np.float32)
    proj_w = np.asarray(proj_w, dtype=np.float32)
    in_maps = []
    for c in range(8):
        b, g = divmod(c, 2)
        qs = slice(g * DL, (g + 1) * DL)
        ks = slice(D + g * DL, D + (g + 1) * DL)
        vs = slice(2 * D + g * DL, 2 * D + (g + 1) * DL)
        wq = qkv_w[qs]  # [DL, D]
        wk = qkv_w[ks]
        wv = qkv_w[vs]
        bqk = np.concatenate([qkv_b[qs], qkv_b[ks]])  # [2*DL]
        in_maps.append(
            {
                "hT": _pack(hidden_states[b].T, P),
                "wqkT": _pack(np.concatenate([wq, wk], axis=0).T, P),
                "wvT": _pack(wv.T, P),
                "bqk": np.ascontiguousarray(
                    bqk.reshape(2 * DL // P, P).T
                ),
                "bv": np.ascontiguousarray(
                    np.broadcast_to(qkv_b[vs], (P, DL))
                ),
                "wpT": _pack(proj_w[:, g * DL : (g + 1) * DL].T, HD),
                "vones": np.ones((P, (S // 128) * HL), dtype=ml_dtypes.bfloat16),
                "o64": np.ones((1, HD), dtype=np.float32),
            }
        )
    return in_maps


def kernel(hidden_states, qkv_w, qkv_b, proj_w, proj_b):
    from concourse.bass_utils import run_bass_kernel_spmd

    hidden_states = np.asarray(hidden_states, dtype=np.float32)
    proj_b = np.asarray(proj_b, dtype=np.float32)
    B, S, d = hidden_states.shape
    in_maps = shard_inputs(hidden_states, qkv_w, qkv_b, proj_w, S=S)
    nc = _build(S)
    res = run_bass_kernel_spmd(nc, in_maps, core_ids=list(range(8)))
    parts = [r["out"] for r in res.results]
    out = np.stack(
        [parts[2 * b] + parts[2 * b + 1] + proj_b[None, :] for b in range(B)]
    )
    return out.astype(np.float32)
